# revision 30
# baseline (speedup 1.0000x reference)
"""Multi-head attention (B=8, T=2048, C=256, H=4) on 8 NeuronCores.

Data-parallel over batch: core b computes batch element b end-to-end.

Per-core dataflow — everything runs "transposed" so the attention
contraction dims land on SBUF partitions and the big score matrices
never need transposing:

  xT   [C, T]      = PE-transpose of x (cast to bf16 during DMA load)
  qkT  [2C, T]     = w_qk @ xT + b_qk   (q/k for all heads; a head PAIR
                                         occupies the two 64-partition
                                         strips of each 128-row chunk)
  v    [T, H, 65]  = x @ w_v.T + b_v    (natural layout, a ones column
                                         appended per head for sumexp)
  per (q-tile of 512 outer, head-pair inner):
    scoresT[k,q] chunks via K=64 matmuls in PE row groups 0/64
    exp on ScalarE straight out of 3-bank PSUM groups (scale=1/8 fused)
    PV accumulates out2T[65, 512] in PSUM over all 16 k-chunks;
      row 64 = sum(exp) thanks to the ones column
    deferred normalization: unnormalized out2T is copied to yT, sumexp
      rows collected at 32-partition spacing; one batched
      reciprocal_approx_fast per q-tile, K=1 ones-matmul broadcasts
      1/sumexp across 64 partitions, one DVE multiply per head
    proj: out[t-chunk, :] = yT[:, t-chunk].T @ w_pT + b_p — yT is the
      STATIONARY operand, so the output lands in natural [T, C] layout
      with no final transpose; bias added via a partition-broadcast
      DMA-loaded tile; stored straight to DRAM.

Dtypes: q/k/v/exp/proj matmuls in bf16 (1 cyc/row with a real,
dedupable LDWEIGHTS; fp32 is 4 cyc/row and float32r self-loads weights
serially per matmul). Scores/PSUM accumulation stay fp32. Softmax skips
max-subtraction: logits are ~N(0, 1/3) so exp() is safely in range.
Measured: ~221 us HW exec, ~3e-3 max rel err vs the fp32 reference.
"""

import numpy as np

import concourse.bass as bass
import concourse.tile as tile
from concourse import bacc, mybir
from concourse.bass_utils import run_bass_kernel_spmd
from concourse.masks import make_identity

B, T, C = 8, 2048, 256
H, HD = 4, 64
N_CORES = 8
F32 = mybir.dt.float32
F32R = mybir.dt.float32r
BF16 = mybir.dt.bfloat16

QT = 512                # q-tile (columns of scoresT per inner iteration)
NQT = T // QT           # 4
KC = T // 128           # 16 k-chunks of 128
GROUP = 3               # score chunks (psum banks) per exp instruction


def build_nc():
    nc = bacc.Bacc("TRN2", target_bir_lowering=False, debug=False,
                   num_devices=N_CORES)

    x_ap = nc.dram_tensor("x", [T, C], F32, kind="ExternalInput").ap()
    wqk_ap = nc.dram_tensor("w_qkT", [C, 2 * C], F32R, kind="ExternalInput").ap()
    wv_ap = nc.dram_tensor("w_vT", [C, C], F32R, kind="ExternalInput").ap()
    wp_ap = nc.dram_tensor("w_pT", [C, C], F32R, kind="ExternalInput").ap()
    bqk_ap = nc.dram_tensor("b_qk", [4, 128], F32, kind="ExternalInput").ap()
    bv_ap = nc.dram_tensor("b_v", [C], F32, kind="ExternalInput").ap()
    bp_ap = nc.dram_tensor("b_p", [C], F32, kind="ExternalInput").ap()
    out_ap = nc.dram_tensor("out", [T, C], F32, kind="ExternalOutput").ap()

    with tile.TileContext(nc) as tc:
        with (
            tc.tile_pool(name="consts", bufs=1) as consts,
            tc.tile_pool(name="xstage", bufs=4) as xstage,
            tc.tile_pool(name="xt", bufs=1) as xtp,
            tc.tile_pool(name="qkt", bufs=1) as qktp,
            tc.tile_pool(name="vsb", bufs=1) as vsbp,
            tc.tile_pool(name="expp", bufs=3) as expp,
            tc.tile_pool(name="yt", bufs=1) as ytp,
            tc.tile_pool(name="outt", bufs=1) as outtp,
            tc.tile_pool(name="ostage", bufs=3) as ostage,
            tc.tile_pool(name="small", bufs=4) as small,
            tc.tile_pool(name="scps", bufs=2, space="PSUM") as scps,
            tc.tile_pool(name="o2ps", bufs=1, space="PSUM") as o2ps,
        ):
            # ---- constants / weights -------------------------------------
            ident = consts.tile([128, 128], BF16, tag="ident")
            make_identity(nc, ident[:])

            ones_f = consts.tile([97, 64], F32, tag="ones_f")
            nc.vector.memset(ones_f[:], 1.0)
            ones_r = consts.tile([97, 64], F32R, tag="ones_r")
            nc.vector.tensor_copy(ones_r[:], ones_f[:])

            onescol = consts.tile([128, H], F32, tag="onescol")
            nc.vector.memset(onescol[:], 1.0)

            w_qk = [consts.tile([128, 2 * C], BF16, tag=f"wqk{c}", name=f"wqk{c}") for c in range(2)]
            for c in range(2):
                nc.gpsimd.dma_start(w_qk[c][:], wqk_ap[128 * c:128 * (c + 1), :])
            w_v = [consts.tile([128, C], BF16, tag=f"wv{c}", name=f"wv{c}") for c in range(2)]
            for c in range(2):
                nc.gpsimd.dma_start(w_v[c][:], wv_ap[128 * c:128 * (c + 1), :])
            w_p = [consts.tile([128, C], BF16, tag=f"wp{c}", name=f"wp{c}") for c in range(2)]
            for c in range(2):
                nc.gpsimd.dma_start(w_p[c][:], wp_ap[128 * c:128 * (c + 1), :])

            b_qk = consts.tile([128, 4], F32, tag="bqk")
            nc.gpsimd.dma_start(b_qk[:], bqk_ap.rearrange("c p -> p c"))
            b_p = consts.tile([128, C], F32, tag="bp")
            bp_bc = bass.AP(tensor=bp_ap.tensor, offset=bp_ap.offset,
                            ap=[[0, 128]] + list(bp_ap.ap))
            nc.gpsimd.dma_start(b_p[:], bp_bc)
            b_v = consts.tile([128, C], F32, tag="bv")
            bv_bc = bass.AP(tensor=bv_ap.tensor, offset=bv_ap.offset,
                            ap=[[0, 128]] + list(bv_ap.ap))
            nc.gpsimd.dma_start(b_v[:], bv_bc)

            # ---- stage A: cast-load x to bf16, PE-transpose to xT --------
            xt = [xtp.tile([128, T], BF16, tag=f"xt{c}", name=f"xt{c}") for c in range(2)]
            x_re = x_ap.rearrange("(b a p) c -> b p a c", b=4, p=128)
            xsbig = [None] * 4
            for b in range(4):
                xsbig[b] = xstage.tile([128, 4, C], BF16, tag="xs", name=f"xs{b}")
                nc.gpsimd.dma_start(xsbig[b][:], x_re[b])
            for tt in range(KC):
                xs = xsbig[tt // 4][:, tt % 4, :]
                for c in range(2):
                    if c == 0:
                        ps = scps.tile([128, 128], BF16, tag="sc", name="tp0")
                    else:
                        ps = o2ps.tile([128, 128], BF16, tag=f"o2{tt % 2}",
                                       name=f"tp{tt % 2}")
                    nc.tensor.transpose(ps[:], xs[:, 128 * c:128 * (c + 1)], ident[:])
                    nc.vector.tensor_copy(xt[c][:, 128 * tt:128 * (tt + 1)], ps[:])

            # ---- stage B: qkT [2C, T] = w_qk.T @ xT + b_qk ---------------
            # m-chunk 0: heads 0,1 q | 1: heads 2,3 q | 2: heads 0,1 k | 3: heads 2,3 k
            # n-outer so attention on q-tile 0 can start after n=0.
            qkt = [qktp.tile([128, T], BF16, tag=f"qkt{m}", name=f"qkt{m}") for m in range(4)]
            def stage_b(n):
                for m in (0, 2, 1, 3):
                    if m % 2 == 0:
                        ps = scps.tile([128, QT], F32, tag="sc", name="bps0")
                    else:
                        ps = o2ps.tile([128, QT], F32, tag=f"o2{m // 2}",
                                       name=f"bps{m}")
                    for c in range(2):
                        nc.tensor.matmul(
                            ps[:], w_qk[c][:, 128 * m:128 * (m + 1)],
                            xt[c][:, QT * n:QT * (n + 1)],
                            start=(c == 0), stop=(c == 1))
                    nc.vector.tensor_scalar_add(
                        qkt[m][:, QT * n:QT * (n + 1)], ps[:], b_qk[:, m:m + 1])

            # ---- stage C: v [T, H, 65] natural + bias + ones column ------
            vsb = [vsbp.tile([128, H, HD + 1], BF16, tag=f"v{tt}", name=f"v{tt}") for tt in range(KC)]
            for tt in range(KC):
                ps = scps.tile([128, C], F32, tag="sc")
                for c in range(2):
                    nc.tensor.matmul(
                        ps[:], xt[c][:, 128 * tt:128 * (tt + 1)], w_v[c][:],
                        start=(c == 0), stop=(c == 1))
                nc.vector.tensor_add(
                    vsb[tt][:, :, 0:HD],
                    ps[:].rearrange("p (h d) -> p h d", h=H),
                    b_v[:].rearrange("p (h d) -> p h d", h=H))
                nc.vector.tensor_copy(
                    vsb[tt][:, :, HD:HD + 1], onescol[:].rearrange("p (h o) -> p h o", o=1))

            for n in range(NQT):
                stage_b(n)

            # ---- stage D: attention, qt outer / head-pair inner ----------
            # yt holds UNNORMALIZED out2T; sumexp rows are collected at
            # 32-partition spacing, normalized in one batched reciprocal per
            # q-tile, broadcast via K=1 matmuls, and multiplied in afterwards.
            yt = [ytp.tile([128, T], BF16, tag=f"yt{hp}", name=f"yt{hp}") for hp in range(2)]
            nsteps = 2 * KC                      # (chunk, head) pairs: 32
            ngroups = (nsteps + GROUP - 1) // GROUP
            for qt in range(NQT):
                se = small.tile([97, QT], F32, tag="se")
                for hp in range(2):
                    qT = qkt[hp]
                    kT = qkt[hp + 2]
                    o2 = [o2ps.tile([HD + 1, QT], F32, tag=f"o2{h}", name=f"o2{h}") for h in range(2)]
                    for g in range(ngroups):
                        ns = min(GROUP, nsteps - GROUP * g)
                        sc = scps.tile([128, GROUP * QT], F32, tag="sc")
                        for s in range(ns):
                            i, h = divmod(GROUP * g + s, 2)
                            nc.tensor.matmul(
                                sc[:, QT * s:QT * (s + 1)],
                                kT[64 * h:64 * (h + 1), 128 * i:128 * (i + 1)],
                                qT[64 * h:64 * (h + 1), QT * qt:QT * (qt + 1)],
                                start=True, stop=True)
                        ex = expp.tile([128, GROUP * QT], BF16, tag="ex")
                        nc.scalar.activation(
                            ex[:, :QT * ns], sc[:, :QT * ns],
                            mybir.ActivationFunctionType.Exp,
                            bias=0.0, scale=float(HD) ** -0.5)
                        for s in range(ns):
                            i, h = divmod(GROUP * g + s, 2)
                            nc.tensor.matmul(
                                o2[h][:],
                                vsb[i][:, 2 * hp + h, :],
                                ex[:, QT * s:QT * (s + 1)],
                                start=(i == 0), stop=(i == KC - 1))
                    for h in range(2):
                        nc.vector.tensor_copy(
                            yt[hp][64 * h:64 * (h + 1), QT * qt:QT * (qt + 1)],
                            o2[h][0:HD, :])
                        nc.vector.tensor_copy(
                            se[32 * (2 * hp + h):32 * (2 * hp + h) + 1, :],
                            o2[h][HD:HD + 1, :])
                rec_f = small.tile([97, QT], F32, tag="rec_f")
                nc.vector.reciprocal_approx_fast(rec_f[:], se[:])
                rec = small.tile([97, QT], F32R, tag="rec")
                nc.vector.tensor_copy(rec[:], rec_f[:])
                for hp in range(2):
                    for h in range(2):
                        p = 32 * (2 * hp + h)
                        bc = o2ps.tile([HD, QT], F32, tag=f"o2{h}", name=f"bc{h}")
                        nc.tensor.matmul(bc[:], ones_r[p:p + 1, :], rec[p:p + 1, :],
                                         start=True, stop=True,
                                         tile_position=(p, 0))
                        ys = yt[hp][64 * h:64 * (h + 1), QT * qt:QT * (qt + 1)]
                        nc.vector.tensor_mul(ys, ys, bc[:])
                # ---- proj for this q-tile: out[t,:] = yT[:,t].T @ w_pT + b_p
                for tt in range(qt * QT // 128, (qt + 1) * QT // 128):
                    ps = o2ps.tile([128, C], F32, tag=f"o2{tt % 2}", name=f"pps{tt % 2}")
                    for c in range(2):
                        nc.tensor.matmul(
                            ps[:], yt[c][:, 128 * tt:128 * (tt + 1)], w_p[c][:],
                            start=(c == 0), stop=(c == 1))
                    ost = ostage.tile([128, C], F32, tag="ost")
                    nc.vector.tensor_add(ost[:], ps[:], b_p[:])
                    nc.sync.dma_start(out_ap[128 * tt:128 * (tt + 1), :], ost[:])
    nc.compile()
    return nc


_NC_CACHE = []


def _get_nc():
    if not _NC_CACHE:
        _NC_CACHE.append(build_nc())
    return _NC_CACHE[0]


def make_in_maps(x, w_qkv, b_qkv, w_proj, b_proj):
    shared = {
        "w_qkT": np.ascontiguousarray(w_qkv[:2 * C].T, dtype=np.float32),
        "w_vT": np.ascontiguousarray(w_qkv[2 * C:].T, dtype=np.float32),
        "w_pT": np.ascontiguousarray(w_proj.T, dtype=np.float32),
        "b_qk": np.ascontiguousarray(b_qkv[:2 * C].reshape(4, 128), dtype=np.float32),
        "b_v": np.ascontiguousarray(b_qkv[2 * C:], dtype=np.float32),
        "b_p": np.ascontiguousarray(b_proj, dtype=np.float32),
    }
    return [dict(shared, x=np.ascontiguousarray(x[b], dtype=np.float32))
            for b in range(B)]


def run(x, w_qkv, b_qkv, w_proj, b_proj, trace=False):
    nc = _get_nc()
    in_maps = make_in_maps(np.asarray(x), np.asarray(w_qkv), np.asarray(b_qkv),
                           np.asarray(w_proj), np.asarray(b_proj))
    res = run_bass_kernel_spmd(nc, in_maps, list(range(N_CORES)), trace=trace)
    out = np.stack([res.results[b]["out"] for b in range(B)])
    return out, res


def kernel(x, w_qkv, b_qkv, w_proj, b_proj):
    out, _ = run(x, w_qkv, b_qkv, w_proj, b_proj, trace=False)
    return out



# revision 31
# speedup vs baseline: 1.0006x; 1.0006x over previous
"""Multi-head attention (B=8, T=2048, C=256, H=4) on 8 NeuronCores.

Data-parallel over batch: core b computes batch element b end-to-end.

Per-core dataflow — everything runs "transposed" so the attention
contraction dims land on SBUF partitions and the big score matrices
never need transposing:

  xT   [C, T]      = PE-transpose of x (cast to bf16 during DMA load)
  qkT  [2C, T]     = w_qk @ xT + b_qk   (q/k for all heads; a head PAIR
                                         occupies the two 64-partition
                                         strips of each 128-row chunk)
  v    [T, H, 65]  = x @ w_v.T + b_v    (natural layout, a ones column
                                         appended per head for sumexp)
  per (q-tile of 512 outer, head-pair inner):
    scoresT[k,q] chunks via K=64 matmuls in PE row groups 0/64
    exp on ScalarE straight out of 3-bank PSUM groups (scale=1/8 fused)
    PV accumulates out2T[65, 512] in PSUM over all 16 k-chunks;
      row 64 = sum(exp) thanks to the ones column
    deferred normalization: unnormalized out2T is copied to yT, sumexp
      rows collected at 32-partition spacing; one batched
      reciprocal_approx_fast per q-tile, K=1 ones-matmul broadcasts
      1/sumexp across 64 partitions, one DVE multiply per head
    proj: out[t-chunk, :] = yT[:, t-chunk].T @ w_pT + b_p — yT is the
      STATIONARY operand, so the output lands in natural [T, C] layout
      with no final transpose; bias added via a partition-broadcast
      DMA-loaded tile; stored straight to DRAM.

Dtypes: q/k/v/exp/proj matmuls in bf16 (1 cyc/row with a real,
dedupable LDWEIGHTS; fp32 is 4 cyc/row and float32r self-loads weights
serially per matmul). Scores/PSUM accumulation stay fp32. Softmax skips
max-subtraction: logits are ~N(0, 1/3) so exp() is safely in range.
Measured: ~221 us HW exec, ~3e-3 max rel err vs the fp32 reference.
"""

import numpy as np

import concourse.bass as bass
import concourse.tile as tile
from concourse import bacc, mybir
from concourse.bass_utils import run_bass_kernel_spmd
from concourse.masks import make_identity

B, T, C = 8, 2048, 256
H, HD = 4, 64
N_CORES = 8
F32 = mybir.dt.float32
F32R = mybir.dt.float32r
BF16 = mybir.dt.bfloat16

QT = 512                # q-tile (columns of scoresT per inner iteration)
NQT = T // QT           # 4
KC = T // 128           # 16 k-chunks of 128
GROUP = 3               # score chunks (psum banks) per exp instruction


def build_nc():
    nc = bacc.Bacc("TRN2", target_bir_lowering=False, debug=False,
                   num_devices=N_CORES)

    x_ap = nc.dram_tensor("x", [T, C], F32, kind="ExternalInput").ap()
    wqk_ap = nc.dram_tensor("w_qkT", [C, 2 * C], F32R, kind="ExternalInput").ap()
    wv_ap = nc.dram_tensor("w_vT", [C, C], F32R, kind="ExternalInput").ap()
    wp_ap = nc.dram_tensor("w_pT", [C, C], F32R, kind="ExternalInput").ap()
    bqk_ap = nc.dram_tensor("b_qk", [4, 128], F32, kind="ExternalInput").ap()
    bv_ap = nc.dram_tensor("b_v", [C], F32, kind="ExternalInput").ap()
    bp_ap = nc.dram_tensor("b_p", [C], F32, kind="ExternalInput").ap()
    out_ap = nc.dram_tensor("out", [T, C], F32, kind="ExternalOutput").ap()

    with tile.TileContext(nc) as tc:
        with (
            tc.tile_pool(name="consts", bufs=1) as consts,
            tc.tile_pool(name="xstage", bufs=4) as xstage,
            tc.tile_pool(name="xt", bufs=1) as xtp,
            tc.tile_pool(name="qkt", bufs=1) as qktp,
            tc.tile_pool(name="vsb", bufs=1) as vsbp,
            tc.tile_pool(name="expp", bufs=3) as expp,
            tc.tile_pool(name="yt", bufs=1) as ytp,
            tc.tile_pool(name="outt", bufs=1) as outtp,
            tc.tile_pool(name="ostage", bufs=3) as ostage,
            tc.tile_pool(name="small", bufs=4) as small,
            tc.tile_pool(name="scps", bufs=2, space="PSUM") as scps,
            tc.tile_pool(name="o2ps", bufs=1, space="PSUM") as o2ps,
        ):
            # ---- constants / weights -------------------------------------
            ident = consts.tile([128, 128], BF16, tag="ident")
            make_identity(nc, ident[:])

            ones_f = consts.tile([97, 64], F32, tag="ones_f")
            nc.vector.memset(ones_f[:], 1.0)
            ones_r = consts.tile([97, 64], F32R, tag="ones_r")
            nc.vector.tensor_copy(ones_r[:], ones_f[:])

            onescol = consts.tile([128, H], F32, tag="onescol")
            nc.vector.memset(onescol[:], 1.0)

            w_qk = [consts.tile([128, 2 * C], BF16, tag=f"wqk{c}", name=f"wqk{c}") for c in range(2)]
            for c in range(2):
                nc.gpsimd.dma_start(w_qk[c][:], wqk_ap[128 * c:128 * (c + 1), :])
            w_v = [consts.tile([128, C], BF16, tag=f"wv{c}", name=f"wv{c}") for c in range(2)]
            for c in range(2):
                nc.gpsimd.dma_start(w_v[c][:], wv_ap[128 * c:128 * (c + 1), :])
            w_p = [consts.tile([128, C], BF16, tag=f"wp{c}", name=f"wp{c}") for c in range(2)]
            for c in range(2):
                nc.gpsimd.dma_start(w_p[c][:], wp_ap[128 * c:128 * (c + 1), :])

            b_qk = consts.tile([128, 4], F32, tag="bqk")
            nc.gpsimd.dma_start(b_qk[:], bqk_ap.rearrange("c p -> p c"))
            b_p = consts.tile([128, C], F32, tag="bp")
            bp_bc = bass.AP(tensor=bp_ap.tensor, offset=bp_ap.offset,
                            ap=[[0, 128]] + list(bp_ap.ap))
            nc.gpsimd.dma_start(b_p[:], bp_bc)
            b_v = consts.tile([128, C], F32, tag="bv")
            bv_bc = bass.AP(tensor=bv_ap.tensor, offset=bv_ap.offset,
                            ap=[[0, 128]] + list(bv_ap.ap))
            nc.gpsimd.dma_start(b_v[:], bv_bc)

            # ---- stage A: cast-load x to bf16, PE-transpose to xT --------
            xt = [xtp.tile([128, T], BF16, tag=f"xt{c}", name=f"xt{c}") for c in range(2)]
            x_re = x_ap.rearrange("(b a p) c -> b p a c", b=4, p=128)
            xsbig = [None] * 4
            for b in range(4):
                xsbig[b] = xstage.tile([128, 4, C], BF16, tag="xs", name=f"xs{b}")
                nc.gpsimd.dma_start(xsbig[b][:], x_re[b])
            for tt in range(KC):
                xs = xsbig[tt // 4][:, tt % 4, :]
                for c in range(2):
                    if c == 0:
                        ps = scps.tile([128, 128], BF16, tag="sc", name="tp0")
                    else:
                        ps = o2ps.tile([128, 128], BF16, tag=f"o2{tt % 2}",
                                       name=f"tp{tt % 2}")
                    nc.tensor.transpose(ps[:], xs[:, 128 * c:128 * (c + 1)], ident[:])
                    nc.vector.tensor_copy(xt[c][:, 128 * tt:128 * (tt + 1)], ps[:])

            # ---- stage B: qkT [2C, T] = w_qk.T @ xT + b_qk ---------------
            # m-chunk 0: heads 0,1 q | 1: heads 2,3 q | 2: heads 0,1 k | 3: heads 2,3 k
            # n-outer so attention on q-tile 0 can start after n=0.
            qkt = [qktp.tile([128, T], BF16, tag=f"qkt{m}", name=f"qkt{m}") for m in range(4)]
            def stage_b(n):
                for m in (0, 2, 1, 3):
                    if m % 2 == 0:
                        ps = scps.tile([128, QT], F32, tag="sc", name="bps0")
                    else:
                        ps = o2ps.tile([128, QT], F32, tag=f"o2{m // 2}",
                                       name=f"bps{m}")
                    for c in range(2):
                        nc.tensor.matmul(
                            ps[:], w_qk[c][:, 128 * m:128 * (m + 1)],
                            xt[c][:, QT * n:QT * (n + 1)],
                            start=(c == 0), stop=(c == 1))
                    nc.vector.tensor_scalar_add(
                        qkt[m][:, QT * n:QT * (n + 1)], ps[:], b_qk[:, m:m + 1])

            # ---- stage C: v [T, H, 65] natural + bias + ones column ------
            vsb = [vsbp.tile([128, H, HD + 1], BF16, tag=f"v{tt}", name=f"v{tt}") for tt in range(KC)]
            for tt in range(KC):
                ps = scps.tile([128, C], F32, tag="sc")
                for c in range(2):
                    nc.tensor.matmul(
                        ps[:], xt[c][:, 128 * tt:128 * (tt + 1)], w_v[c][:],
                        start=(c == 0), stop=(c == 1))
                nc.vector.tensor_add(
                    vsb[tt][:, :, 0:HD],
                    ps[:].rearrange("p (h d) -> p h d", h=H),
                    b_v[:].rearrange("p (h d) -> p h d", h=H))
                nc.vector.tensor_copy(
                    vsb[tt][:, :, HD:HD + 1], onescol[:].rearrange("p (h o) -> p h o", o=1))

            for n in range(NQT):
                stage_b(n)

            # ---- stage D: attention, qt outer / head-pair inner ----------
            # yt holds UNNORMALIZED out2T; sumexp rows are collected at
            # 32-partition spacing, normalized in one batched reciprocal per
            # q-tile, broadcast via K=1 matmuls, and multiplied in afterwards.
            yt = [ytp.tile([128, T], BF16, tag=f"yt{hp}", name=f"yt{hp}") for hp in range(2)]
            nsteps = 2 * KC                      # (chunk, head) pairs: 32
            ngroups = (nsteps + GROUP - 1) // GROUP
            for qt in range(NQT):
                se = small.tile([97, QT], F32, tag="se")
                for hp in range(2):
                    qT = qkt[hp]
                    kT = qkt[hp + 2]
                    o2 = [o2ps.tile([HD + 1, QT], F32, tag=f"o2{h}", name=f"o2{h}") for h in range(2)]
                    # software-pipelined emission: QK(g+1) is placed in the
                    # PE stream BEFORE PV(g) so the PE never head-of-line
                    # blocks on exp(g) while QK work is available.
                    def emit_pv(g, ex, ns):
                        for s in range(ns):
                            i, h = divmod(GROUP * g + s, 2)
                            nc.tensor.matmul(
                                o2[h][:],
                                vsb[i][:, 2 * hp + h, :],
                                ex[:, QT * s:QT * (s + 1)],
                                start=(i == 0), stop=(i == KC - 1))
                    prev = None
                    for g in range(ngroups):
                        ns = min(GROUP, nsteps - GROUP * g)
                        sc = scps.tile([128, GROUP * QT], F32, tag="sc")
                        for s in range(ns):
                            i, h = divmod(GROUP * g + s, 2)
                            nc.tensor.matmul(
                                sc[:, QT * s:QT * (s + 1)],
                                kT[64 * h:64 * (h + 1), 128 * i:128 * (i + 1)],
                                qT[64 * h:64 * (h + 1), QT * qt:QT * (qt + 1)],
                                start=True, stop=True)
                        ex = expp.tile([128, GROUP * QT], BF16, tag="ex")
                        nc.scalar.activation(
                            ex[:, :QT * ns], sc[:, :QT * ns],
                            mybir.ActivationFunctionType.Exp,
                            bias=0.0, scale=float(HD) ** -0.5)
                        if prev is not None:
                            emit_pv(*prev)
                        prev = (g, ex, ns)
                    emit_pv(*prev)
                    for h in range(2):
                        nc.vector.tensor_copy(
                            yt[hp][64 * h:64 * (h + 1), QT * qt:QT * (qt + 1)],
                            o2[h][0:HD, :])
                        nc.vector.tensor_copy(
                            se[32 * (2 * hp + h):32 * (2 * hp + h) + 1, :],
                            o2[h][HD:HD + 1, :])
                rec_f = small.tile([97, QT], F32, tag="rec_f")
                nc.vector.reciprocal_approx_fast(rec_f[:], se[:])
                rec = small.tile([97, QT], F32R, tag="rec")
                nc.vector.tensor_copy(rec[:], rec_f[:])
                for hp in range(2):
                    for h in range(2):
                        p = 32 * (2 * hp + h)
                        bc = o2ps.tile([HD, QT], F32, tag=f"o2{h}", name=f"bc{h}")
                        nc.tensor.matmul(bc[:], ones_r[p:p + 1, :], rec[p:p + 1, :],
                                         start=True, stop=True,
                                         tile_position=(p, 0))
                        ys = yt[hp][64 * h:64 * (h + 1), QT * qt:QT * (qt + 1)]
                        nc.vector.tensor_mul(ys, ys, bc[:])
                # ---- proj for this q-tile: out[t,:] = yT[:,t].T @ w_pT + b_p
                for tt in range(qt * QT // 128, (qt + 1) * QT // 128):
                    ps = o2ps.tile([128, C], F32, tag=f"o2{tt % 2}", name=f"pps{tt % 2}")
                    for c in range(2):
                        nc.tensor.matmul(
                            ps[:], yt[c][:, 128 * tt:128 * (tt + 1)], w_p[c][:],
                            start=(c == 0), stop=(c == 1))
                    ost = ostage.tile([128, C], F32, tag="ost")
                    nc.vector.tensor_add(ost[:], ps[:], b_p[:])
                    nc.sync.dma_start(out_ap[128 * tt:128 * (tt + 1), :], ost[:])
    nc.compile()
    return nc


_NC_CACHE = []


def _get_nc():
    if not _NC_CACHE:
        _NC_CACHE.append(build_nc())
    return _NC_CACHE[0]


def make_in_maps(x, w_qkv, b_qkv, w_proj, b_proj):
    shared = {
        "w_qkT": np.ascontiguousarray(w_qkv[:2 * C].T, dtype=np.float32),
        "w_vT": np.ascontiguousarray(w_qkv[2 * C:].T, dtype=np.float32),
        "w_pT": np.ascontiguousarray(w_proj.T, dtype=np.float32),
        "b_qk": np.ascontiguousarray(b_qkv[:2 * C].reshape(4, 128), dtype=np.float32),
        "b_v": np.ascontiguousarray(b_qkv[2 * C:], dtype=np.float32),
        "b_p": np.ascontiguousarray(b_proj, dtype=np.float32),
    }
    return [dict(shared, x=np.ascontiguousarray(x[b], dtype=np.float32))
            for b in range(B)]


def run(x, w_qkv, b_qkv, w_proj, b_proj, trace=False):
    nc = _get_nc()
    in_maps = make_in_maps(np.asarray(x), np.asarray(w_qkv), np.asarray(b_qkv),
                           np.asarray(w_proj), np.asarray(b_proj))
    res = run_bass_kernel_spmd(nc, in_maps, list(range(N_CORES)), trace=trace)
    out = np.stack([res.results[b]["out"] for b in range(B)])
    return out, res


def kernel(x, w_qkv, b_qkv, w_proj, b_proj):
    out, _ = run(x, w_qkv, b_qkv, w_proj, b_proj, trace=False)
    return out

